# revision 9
# baseline (speedup 1.0000x reference)
"""JSONTreeLSTM Trainium2 kernel: 8-core data-parallel over K=4096 array children.

Layout: transposed — [128 partitions = mem/gate dims, K_loc=512 free = array index].
The number-embedding + running-stat normalization collapses algebraically into the
gate computation: gates = W_hh @ h + u' (x) x_raw_t + v', with
u' = s_c * (W_ih[:,128:] @ w_num), v' = W_ih[:,128:] @ b_num + b_ih + b_hh - m_c*u'
(s_c, m_c = the post-cap running stats, constant for all flat indices >= 100).
The 100 prefix-normalized elements (flat idx < 100 = numbers[0, :100], core 0 only)
are patched into x via x_eff = x_norm/s_c + m_c so the same affine maps them right.

Scan step (128 steps):
  DMA: stage x row t [1, 512] from DRAM to partition 0
  PE:  per gate g: psum_g = W_hh_g @ h.T + rank-1 u'_g (x) x_t
  ACT: sigma_g = Sigmoid(psum_g + v'_g)  (g-gate pre-scaled x2: tanh(g)=2*sig(2g)-1)
  DVE: w=2*sg2-1; m1=sf*c; m2=si*w; c2=m1+m2; ACT sig(2*c2); DVE w2=2*s-1; h2=so*w2
Root: sigmoid(W_fh h + b_fh)*c and h child-sums -> AllReduce -> tree-LSTM root.
"""
import sys

sys.path.insert(0, "/opt/trn_rl_repo")
import numpy as np
import concourse.bacc as bacc
import concourse.mybir as mybir
import concourse.tile as tile
from concourse import bass_utils

F32 = mybir.dt.float32
AF = mybir.ActivationFunctionType
OP = mybir.AluOpType
AX = mybir.AxisListType

K, L, MEM, NCORES = 4096, 128, 128, 8
KLOC = K // NCORES  # 512
STATS_CAP = 100

_compiled = {}


def _build(n_cores=NCORES):
    nc = bacc.Bacc("TRN2", target_bir_lowering=False, debug=False,
                   num_devices=n_cores)

    def din(name, shape):
        return nc.dram_tensor(name, shape, F32, kind="ExternalInput").ap()

    BF16 = mybir.dt.bfloat16
    xT_d = nc.dram_tensor("xT", [L, 2, KLOC], BF16,
                          kind="ExternalInput").ap()  # per t: [x_eff_t; ones]
    whhT_d = nc.dram_tensor("whhT", [MEM, 4 * MEM], BF16,
                            kind="ExternalInput").ap()  # W_hh.T, g-block x2
    uvrow_d = nc.dram_tensor("uvrow", [2, 4 * MEM], BF16,
                             kind="ExternalInput").ap()  # rows u', v', g-block x2
    wfhT_d = nc.dram_tensor("wfhT", [MEM, MEM], BF16, kind="ExternalInput").ap()
    bfh_d = din("bfh", [MEM, 1])
    wiouhT_d = din("wiouhT", [MEM, 3 * MEM])
    biouh_d = din("biouh", [MEM, 3])
    wloutT_d = din("wloutT", [MEM, MEM])
    blout_d = din("blout", [MEM, 1])
    out_d = nc.dram_tensor("out", [MEM, 2], F32, kind="ExternalOutput").ap()

    with tile.TileContext(nc) as tc:
        with tc.tile_pool(name="const", bufs=1) as cp, \
             tc.tile_pool(name="state", bufs=2) as sp, \
             tc.tile_pool(name="xrow", bufs=6) as xp, \
             tc.tile_pool(name="psum", bufs=2, space="PSUM") as pp, \
             tc.tile_pool(name="dram", bufs=1, space="DRAM") as dp:

            whhT = cp.tile([MEM, 4 * MEM], BF16, tag="whhT")
            uvrow = cp.tile([2, 4 * MEM], BF16, tag="uvrow")
            wfhT = cp.tile([MEM, MEM], BF16, tag="wfhT")
            bfh = cp.tile([MEM, 1], F32, tag="bfh")
            wiouhT = cp.tile([MEM, 3 * MEM], F32, tag="wiouhT")
            biouh = cp.tile([MEM, 3], F32, tag="biouh")
            wloutT = cp.tile([MEM, MEM], F32, tag="wloutT")
            blout = cp.tile([MEM, 1], F32, tag="blout")
            for t, d in [(whhT, whhT_d), (uvrow, uvrow_d),
                         (wfhT, wfhT_d), (bfh, bfh_d), (wiouhT, wiouhT_d),
                         (biouh, biouh_d), (wloutT, wloutT_d), (blout, blout_d)]:
                nc.sync.dma_start(t[:], d[:])

            # ---- LSTM scan: 2 independent k-chains hide the serial latency ----
            CH = 2
            KH = KLOC // CH
            h = []
            c = []
            for a in range(CH):
                ht = sp.tile([MEM, KH], BF16, tag=f"h{a}", name=f"h{a}_init")
                ct = sp.tile([MEM, KH], F32, tag=f"c{a}", name=f"c{a}_init")
                nc.any.memset(ht[:], 0.0)
                nc.any.memset(ct[:], 0.0)
                h.append(ht)
                c.append(ct)

            for t in range(L):
                xr = xp.tile([2, KLOC], BF16, tag="xr", name=f"xr_{t}")
                nc.sync.dma_start(xr[:], xT_d[t, :, :])
                # full-width x-injection: 4 matmuls over all k (both chains)
                gpf = pp.tile([MEM, 4 * KLOC], F32, tag="gpf", name=f"gpf_{t}")
                for j in range(4):
                    nc.tensor.matmul(gpf[:, j * KLOC:(j + 1) * KLOC],
                                     uvrow[:, j * MEM:(j + 1) * MEM],
                                     xr[:], start=True, stop=False)
                gpf3 = gpf[:].rearrange("p (g k) -> p g k", g=4)
                for a in range(CH):
                    ks = slice(a * KH, (a + 1) * KH)
                    sg = sp.tile([MEM, 4 * KH], BF16, tag=f"sg{a}",
                                 name=f"sg{a}_{t}")
                    for j in range(4):
                        nc.tensor.matmul(gpf[:, j * KLOC + a * KH:
                                             j * KLOC + (a + 1) * KH],
                                         whhT[:, j * MEM:(j + 1) * MEM],
                                         h[a][:], start=False, stop=True)
                    sg3 = sg[:].rearrange("p (g k) -> p g k", g=4)
                    nc.scalar.activation(sg3, gpf3[:, :, ks], AF.Sigmoid)
                    si = sg[:, 0:KH]
                    sf = sg[:, KH:2 * KH]
                    sg2 = sg[:, 2 * KH:3 * KH]
                    so = sg[:, 3 * KH:4 * KH]
                    w = sp.tile([MEM, KH], BF16, tag=f"w{a}", name=f"w{a}_{t}")
                    m1 = sp.tile([MEM, KH], F32, tag=f"m1{a}", name=f"m1{a}_{t}")
                    c2 = sp.tile([MEM, KH], F32, tag=f"c{a}", name=f"c{a}_{t}")
                    nc.vector.tensor_scalar(w, sg2, 2.0, -1.0,
                                            op0=OP.mult, op1=OP.add)
                    nc.vector.tensor_mul(m1, sf, c[a][:])
                    m2 = sp.tile([MEM, KH], F32, tag=f"m2{a}", name=f"m2{a}_{t}")
                    nc.vector.tensor_mul(m2, si, w)
                    nc.vector.tensor_add(c2, m1, m2)
                    s2c = sp.tile([MEM, KH], BF16, tag=f"s2c{a}",
                                  name=f"s2c{a}_{t}")
                    nc.scalar.activation(s2c, c2[:], AF.Sigmoid, scale=2.0)
                    w2 = sp.tile([MEM, KH], BF16, tag=f"w{a}", name=f"w2{a}_{t}")
                    h2 = sp.tile([MEM, KH], BF16, tag=f"h{a}", name=f"h{a}_{t}")
                    nc.vector.tensor_scalar(w2, s2c, 2.0, -1.0,
                                            op0=OP.mult, op1=OP.add)
                    nc.vector.tensor_mul(h2, so, w2)
                    h[a], c[a] = h2, c2

            # ---- root child-sum ----
            part4 = cp.tile([MEM, 4], F32, tag="part4")
            for a in range(CH):
                fgp = pp.tile([MEM, KH], F32, tag="gpf", name=f"fgp{a}")
                nc.tensor.matmul(fgp[:], wfhT[:], h[a][:], start=True, stop=True)
                fg = sp.tile([MEM, KH], F32, tag=f"m1{a}", name=f"fg{a}")
                nc.scalar.activation(fg, fgp[:], AF.Sigmoid, bias=bfh[:])
                fc = sp.tile([MEM, KH], F32, tag=f"w{a}", name=f"fc{a}")
                nc.vector.tensor_mul(fc, fg, c[a][:])
                nc.vector.reduce_sum(part4[:, a:a + 1], fc, axis=AX.X)
                nc.vector.reduce_sum(part4[:, 2 + a:3 + a], h[a][:], axis=AX.X)
            part = cp.tile([MEM, 2], F32, tag="part")
            nc.vector.tensor_add(part[:, 0:1], part4[:, 0:1], part4[:, 1:2])
            nc.vector.tensor_add(part[:, 1:2], part4[:, 2:3], part4[:, 3:4])

            bin_ = dp.tile([MEM, 2], F32)
            bout = dp.tile([MEM, 2], F32)
            nc.sync.dma_start(bin_[:], part[:])
            nc.gpsimd.collective_compute(
                "AllReduce", OP.add,
                replica_groups=[list(range(n_cores))],
                ins=[bin_.opt()], outs=[bout.opt()])
            red = cp.tile([MEM, 2], F32, tag="red")
            nc.sync.dma_start(red[:], bout[:])
            fcsum = red[:, 0:1]
            hbar = red[:, 1:2]

            # ---- root tree-LSTM ----
            ioup = pp.tile([MEM, 3], F32, tag="gpf")
            for j in range(3):
                nc.tensor.matmul(ioup[:, j:j + 1], wiouhT[:, j * MEM:(j + 1) * MEM],
                                 hbar, start=True, stop=True)
            rr = cp.tile([MEM, 8], F32, tag="rr")
            i_r = rr[:, 0:1]
            o_r = rr[:, 1:2]
            u_r = rr[:, 2:3]
            nc.scalar.activation(i_r, ioup[:, 0:1], AF.Sigmoid, bias=biouh[:, 0:1])
            nc.scalar.activation(o_r, ioup[:, 1:2], AF.Sigmoid, bias=biouh[:, 1:2])
            nc.scalar.activation(u_r, ioup[:, 2:3], AF.Tanh, bias=biouh[:, 2:3])
            cr = rr[:, 3:4]
            nc.vector.tensor_mul(cr, i_r, u_r)
            nc.vector.tensor_add(cr, cr, fcsum)
            tcr = rr[:, 4:5]
            nc.scalar.activation(tcr, cr, AF.Tanh)
            hr = rr[:, 5:6]
            nc.vector.tensor_mul(hr, o_r, tcr)
            hhp = pp.tile([MEM, 1], F32, tag="gpf")
            nc.tensor.matmul(hhp[:], wloutT[:], hr, start=True, stop=True)
            outs = cp.tile([MEM, 2], F32, tag="outs")
            nc.vector.tensor_copy(outs[:, 0:1], cr)
            nc.vector.tensor_scalar_add(outs[:, 1:2], hhp[:], blout[:])
            nc.sync.dma_start(out_d[:], outs[:])

    nc.compile()
    return nc


def _prep_inputs(numbers, w_num, b_num, W_ih, W_hh, b_ih, b_hh,
                 W_fh, b_fh, W_iouh, b_iouh, W_lout, b_lout):
    f = np.float32
    numbers = np.ascontiguousarray(numbers, f)

    # Running-stat normalization (reference semantics), first STATS_CAP elems.
    x100 = numbers.reshape(-1)[:STATS_CAP].astype(f)
    kk = np.arange(1, STATS_CAP + 1, dtype=f)
    cs = np.cumsum(x100, dtype=f)
    css = np.cumsum(x100 * x100, dtype=f)
    mean_k = cs / kk
    var_k = np.maximum(css / kk - mean_k * mean_k, 0.0)
    std_k = np.sqrt(var_k)
    use_k = (kk > 3.0) & (std_k > 1e-8)
    inv_k = np.where(use_k, 1.0 / np.where(use_k, std_k, 1.0), 1.0).astype(f)
    x_norm0 = (x100 - mean_k) * inv_k
    m_c = float(mean_k[-1])
    s_c = float(inv_k[-1])

    Wr = np.asarray(W_ih, f)[:, MEM:]                      # [512, 128]
    u = (Wr @ np.asarray(w_num, f)) * s_c
    v = (Wr @ np.asarray(b_num, f) + np.asarray(b_ih, f)
         + np.asarray(b_hh, f) - m_c * u)
    whhT = np.asarray(W_hh, f).T.copy()                    # [128, 512]
    whhT[:, 2 * MEM:3 * MEM] *= 2.0                        # g-gate -> 2g
    u = u.astype(f).copy()
    v = v.astype(f).copy()
    u[2 * MEM:3 * MEM] *= 2.0
    v[2 * MEM:3 * MEM] *= 2.0

    try:
        import ml_dtypes
        bf16 = ml_dtypes.bfloat16
    except ImportError:
        import jax.numpy as jnp
        bf16 = jnp.bfloat16
    shared = {
        "whhT": whhT.astype(bf16),
        "uvrow": np.stack([u, v]).astype(bf16),
        "wfhT": np.asarray(W_fh, f).T.copy().astype(bf16),
        "bfh": np.asarray(b_fh, f).reshape(MEM, 1).copy(),
        "wiouhT": np.asarray(W_iouh, f).T.copy(),
        "biouh": np.asarray(b_iouh, f).reshape(3, MEM).T.copy(),
        "wloutT": np.asarray(W_lout, f).T.copy(),
        "blout": np.asarray(b_lout, f).reshape(MEM, 1).copy(),
    }
    in_maps = []
    for cid in range(NCORES):
        m = dict(shared)
        xT = np.ascontiguousarray(numbers[cid * KLOC:(cid + 1) * KLOC, :].T, f)
        if cid == 0:
            # patch flat elements < 100 (k=0 column -> x row entries [t, 0])
            # so the constant affine reproduces their prefix normalization
            x_eff = x_norm0 / s_c + m_c
            xT[:STATS_CAP, 0] = x_eff
        xaug = np.ones((L, 2, KLOC), f)
        xaug[:, 0, :] = xT
        m["xT"] = xaug.astype(bf16)
        in_maps.append(m)
    return in_maps


def kernel(**inputs):
    if "nc" not in _compiled:
        _compiled["nc"] = _build()
    nc = _compiled["nc"]
    in_maps = _prep_inputs(**inputs)
    res = bass_utils.run_bass_kernel_spmd(nc, in_maps,
                                          core_ids=list(range(NCORES)))
    out = res.results[0]["out"]                            # [128, 2]
    return np.concatenate([out[:, 0], out[:, 1]])[None, :].astype(np.float32)


# revision 11
# speedup vs baseline: 1.0252x; 1.0252x over previous
"""JSONTreeLSTM Trainium2 kernel: 8-core data-parallel over K=4096 array children.

Layout: transposed — [128 partitions = mem/gate dims, K_loc=512 free = array index].
The number-embedding + running-stat normalization collapses algebraically into the
gate computation: gates = W_hh @ h + u' (x) x_raw_t + v', with
u' = s_c * (W_ih[:,128:] @ w_num), v' = W_ih[:,128:] @ b_num + b_ih + b_hh - m_c*u'
(s_c, m_c = the post-cap running stats, constant for all flat indices >= 100).
The 100 prefix-normalized elements (flat idx < 100 = numbers[0, :100], core 0 only)
are patched into x via x_eff = x_norm/s_c + m_c so the same affine maps them right.

Scan step (128 steps):
  DMA: stage x row t [1, 512] from DRAM to partition 0
  PE:  per gate g: psum_g = W_hh_g @ h.T + rank-1 u'_g (x) x_t
  ACT: sigma_g = Sigmoid(psum_g + v'_g)  (g-gate pre-scaled x2: tanh(g)=2*sig(2g)-1)
  DVE: w=2*sg2-1; m1=sf*c; m2=si*w; c2=m1+m2; ACT sig(2*c2); DVE w2=2*s-1; h2=so*w2
Root: sigmoid(W_fh h + b_fh)*c and h child-sums -> AllReduce -> tree-LSTM root.
"""
import sys

sys.path.insert(0, "/opt/trn_rl_repo")
import numpy as np
import concourse.bacc as bacc
import concourse.mybir as mybir
import concourse.tile as tile
from concourse import bass_utils

F32 = mybir.dt.float32
AF = mybir.ActivationFunctionType
OP = mybir.AluOpType
AX = mybir.AxisListType

K, L, MEM, NCORES = 4096, 128, 128, 8
KLOC = K // NCORES  # 512
STATS_CAP = 100

_compiled = {}


def _build(n_cores=NCORES):
    nc = bacc.Bacc("TRN2", target_bir_lowering=False, debug=False,
                   num_devices=n_cores)

    def din(name, shape):
        return nc.dram_tensor(name, shape, F32, kind="ExternalInput").ap()

    BF16 = mybir.dt.bfloat16
    xT_d = nc.dram_tensor("xT", [L, 2, KLOC], BF16,
                          kind="ExternalInput").ap()  # per t: [x_eff_t; ones]
    whhT_d = nc.dram_tensor("whhT", [MEM, 4 * MEM], BF16,
                            kind="ExternalInput").ap()  # W_hh.T, g-block x2
    uvrow_d = nc.dram_tensor("uvrow", [2, 4 * MEM], BF16,
                             kind="ExternalInput").ap()  # rows u', v', g-block x2
    wfhT_d = nc.dram_tensor("wfhT", [MEM, MEM], BF16, kind="ExternalInput").ap()
    bfh_d = din("bfh", [MEM, 1])
    wiouhT_d = din("wiouhT", [MEM, 3 * MEM])
    biouh_d = din("biouh", [MEM, 3])
    wloutT_d = din("wloutT", [MEM, MEM])
    blout_d = din("blout", [MEM, 1])
    out_d = nc.dram_tensor("out", [MEM, 2], F32, kind="ExternalOutput").ap()

    with tile.TileContext(nc) as tc:
        with tc.tile_pool(name="const", bufs=1) as cp, \
             tc.tile_pool(name="state", bufs=3) as sp, \
             tc.tile_pool(name="xrow", bufs=6) as xp, \
             tc.tile_pool(name="psum", bufs=2, space="PSUM") as pp, \
             tc.tile_pool(name="dram", bufs=1, space="DRAM") as dp:

            whhT = cp.tile([MEM, 4 * MEM], BF16, tag="whhT")
            uvrow = cp.tile([2, 4 * MEM], BF16, tag="uvrow")
            wfhT = cp.tile([MEM, MEM], BF16, tag="wfhT")
            bfh = cp.tile([MEM, 1], F32, tag="bfh")
            wiouhT = cp.tile([MEM, 3 * MEM], F32, tag="wiouhT")
            biouh = cp.tile([MEM, 3], F32, tag="biouh")
            wloutT = cp.tile([MEM, MEM], F32, tag="wloutT")
            blout = cp.tile([MEM, 1], F32, tag="blout")
            for t, d in [(whhT, whhT_d), (uvrow, uvrow_d),
                         (wfhT, wfhT_d), (bfh, bfh_d), (wiouhT, wiouhT_d),
                         (biouh, biouh_d), (wloutT, wloutT_d), (blout, blout_d)]:
                nc.sync.dma_start(t[:], d[:])

            # ---- LSTM scan: 2 independent k-chains hide the serial latency ----
            CH = 2
            KH = KLOC // CH
            h = []
            c = []
            for a in range(CH):
                ht = sp.tile([MEM, KH], BF16, tag=f"h{a}", name=f"h{a}_init")
                ct = sp.tile([MEM, KH], F32, tag=f"c{a}", name=f"c{a}_init")
                nc.any.memset(ht[:], 0.0)
                nc.any.memset(ct[:], 0.0)
                h.append(ht)
                c.append(ct)

            for t in range(L):
                xr = xp.tile([2, KLOC], BF16, tag="xr", name=f"xr_{t}")
                nc.sync.dma_start(xr[:], xT_d[t, :, :])
                # full-width x-injection: 4 matmuls over all k (both chains)
                gpf = pp.tile([MEM, 4 * KLOC], F32, tag="gpf", name=f"gpf_{t}")
                for j in range(4):
                    nc.tensor.matmul(gpf[:, j * KLOC:(j + 1) * KLOC],
                                     uvrow[:, j * MEM:(j + 1) * MEM],
                                     xr[:], start=True, stop=False)
                gpf3 = gpf[:].rearrange("p (g k) -> p g k", g=4)
                for a in range(CH):
                    ks = slice(a * KH, (a + 1) * KH)
                    sg = sp.tile([MEM, 4 * KH], F32, tag=f"sg{a}",
                                 name=f"sg{a}_{t}")
                    for j in range(4):
                        nc.tensor.matmul(gpf[:, j * KLOC + a * KH:
                                             j * KLOC + (a + 1) * KH],
                                         whhT[:, j * MEM:(j + 1) * MEM],
                                         h[a][:], start=False, stop=True)
                    sg3 = sg[:].rearrange("p (g k) -> p g k", g=4)
                    nc.scalar.activation(sg3, gpf3[:, :, ks], AF.Sigmoid)
                    si = sg[:, 0:KH]
                    sf = sg[:, KH:2 * KH]
                    sg2 = sg[:, 2 * KH:3 * KH]
                    so = sg[:, 3 * KH:4 * KH]
                    w = sp.tile([MEM, KH], F32, tag=f"w{a}", name=f"w{a}_{t}")
                    m1 = sp.tile([MEM, KH], F32, tag=f"m1{a}", name=f"m1{a}_{t}")
                    c2 = sp.tile([MEM, KH], F32, tag=f"c{a}", name=f"c{a}_{t}")
                    nc.vector.tensor_scalar(w, sg2, 2.0, -1.0,
                                            op0=OP.mult, op1=OP.add)
                    nc.vector.tensor_mul(m1, sf, c[a][:])
                    nc.vector.tensor_mul(w, si, w)
                    nc.vector.tensor_add(c2, m1, w)
                    s2c = sp.tile([MEM, KH], F32, tag=f"s2c{a}",
                                  name=f"s2c{a}_{t}")
                    nc.scalar.activation(s2c, c2[:], AF.Sigmoid, scale=2.0)
                    w2 = sp.tile([MEM, KH], F32, tag=f"w{a}", name=f"w2{a}_{t}")
                    h2 = sp.tile([MEM, KH], BF16, tag=f"h{a}", name=f"h{a}_{t}")
                    nc.vector.tensor_scalar(w2, s2c, 2.0, -1.0,
                                            op0=OP.mult, op1=OP.add)
                    nc.vector.tensor_mul(h2, so, w2)
                    h[a], c[a] = h2, c2

            # ---- root child-sum ----
            part4 = cp.tile([MEM, 4], F32, tag="part4")
            for a in range(CH):
                fgp = pp.tile([MEM, KH], F32, tag="gpf", name=f"fgp{a}")
                nc.tensor.matmul(fgp[:], wfhT[:], h[a][:], start=True, stop=True)
                fg = sp.tile([MEM, KH], F32, tag=f"sg{a}", name=f"fg{a}")
                nc.scalar.activation(fg, fgp[:], AF.Sigmoid, bias=bfh[:])
                fc = sp.tile([MEM, KH], F32, tag=f"w{a}", name=f"fc{a}")
                nc.vector.tensor_mul(fc, fg, c[a][:])
                nc.vector.reduce_sum(part4[:, a:a + 1], fc, axis=AX.X)
                nc.vector.reduce_sum(part4[:, 2 + a:3 + a], h[a][:], axis=AX.X)
            part = cp.tile([MEM, 2], F32, tag="part")
            nc.vector.tensor_add(part[:, 0:1], part4[:, 0:1], part4[:, 1:2])
            nc.vector.tensor_add(part[:, 1:2], part4[:, 2:3], part4[:, 3:4])

            bin_ = dp.tile([MEM, 2], F32)
            bout = dp.tile([MEM, 2], F32)
            nc.sync.dma_start(bin_[:], part[:])
            nc.gpsimd.collective_compute(
                "AllReduce", OP.add,
                replica_groups=[list(range(n_cores))],
                ins=[bin_.opt()], outs=[bout.opt()])
            red = cp.tile([MEM, 2], F32, tag="red")
            nc.sync.dma_start(red[:], bout[:])
            fcsum = red[:, 0:1]
            hbar = red[:, 1:2]

            # ---- root tree-LSTM ----
            ioup = pp.tile([MEM, 3], F32, tag="gpf")
            for j in range(3):
                nc.tensor.matmul(ioup[:, j:j + 1], wiouhT[:, j * MEM:(j + 1) * MEM],
                                 hbar, start=True, stop=True)
            rr = cp.tile([MEM, 8], F32, tag="rr")
            i_r = rr[:, 0:1]
            o_r = rr[:, 1:2]
            u_r = rr[:, 2:3]
            nc.scalar.activation(i_r, ioup[:, 0:1], AF.Sigmoid, bias=biouh[:, 0:1])
            nc.scalar.activation(o_r, ioup[:, 1:2], AF.Sigmoid, bias=biouh[:, 1:2])
            nc.scalar.activation(u_r, ioup[:, 2:3], AF.Tanh, bias=biouh[:, 2:3])
            cr = rr[:, 3:4]
            nc.vector.tensor_mul(cr, i_r, u_r)
            nc.vector.tensor_add(cr, cr, fcsum)
            tcr = rr[:, 4:5]
            nc.scalar.activation(tcr, cr, AF.Tanh)
            hr = rr[:, 5:6]
            nc.vector.tensor_mul(hr, o_r, tcr)
            hhp = pp.tile([MEM, 1], F32, tag="gpf")
            nc.tensor.matmul(hhp[:], wloutT[:], hr, start=True, stop=True)
            outs = cp.tile([MEM, 2], F32, tag="outs")
            nc.vector.tensor_copy(outs[:, 0:1], cr)
            nc.vector.tensor_scalar_add(outs[:, 1:2], hhp[:], blout[:])
            nc.sync.dma_start(out_d[:], outs[:])

    nc.compile()
    return nc


def _prep_inputs(numbers, w_num, b_num, W_ih, W_hh, b_ih, b_hh,
                 W_fh, b_fh, W_iouh, b_iouh, W_lout, b_lout):
    f = np.float32
    numbers = np.ascontiguousarray(numbers, f)

    # Running-stat normalization (reference semantics), first STATS_CAP elems.
    x100 = numbers.reshape(-1)[:STATS_CAP].astype(f)
    kk = np.arange(1, STATS_CAP + 1, dtype=f)
    cs = np.cumsum(x100, dtype=f)
    css = np.cumsum(x100 * x100, dtype=f)
    mean_k = cs / kk
    var_k = np.maximum(css / kk - mean_k * mean_k, 0.0)
    std_k = np.sqrt(var_k)
    use_k = (kk > 3.0) & (std_k > 1e-8)
    inv_k = np.where(use_k, 1.0 / np.where(use_k, std_k, 1.0), 1.0).astype(f)
    x_norm0 = (x100 - mean_k) * inv_k
    m_c = float(mean_k[-1])
    s_c = float(inv_k[-1])

    Wr = np.asarray(W_ih, f)[:, MEM:]                      # [512, 128]
    u = (Wr @ np.asarray(w_num, f)) * s_c
    v = (Wr @ np.asarray(b_num, f) + np.asarray(b_ih, f)
         + np.asarray(b_hh, f) - m_c * u)
    whhT = np.asarray(W_hh, f).T.copy()                    # [128, 512]
    whhT[:, 2 * MEM:3 * MEM] *= 2.0                        # g-gate -> 2g
    u = u.astype(f).copy()
    v = v.astype(f).copy()
    u[2 * MEM:3 * MEM] *= 2.0
    v[2 * MEM:3 * MEM] *= 2.0

    try:
        import ml_dtypes
        bf16 = ml_dtypes.bfloat16
    except ImportError:
        import jax.numpy as jnp
        bf16 = jnp.bfloat16
    shared = {
        "whhT": whhT.astype(bf16),
        "uvrow": np.stack([u, v]).astype(bf16),
        "wfhT": np.asarray(W_fh, f).T.copy().astype(bf16),
        "bfh": np.asarray(b_fh, f).reshape(MEM, 1).copy(),
        "wiouhT": np.asarray(W_iouh, f).T.copy(),
        "biouh": np.asarray(b_iouh, f).reshape(3, MEM).T.copy(),
        "wloutT": np.asarray(W_lout, f).T.copy(),
        "blout": np.asarray(b_lout, f).reshape(MEM, 1).copy(),
    }
    in_maps = []
    for cid in range(NCORES):
        m = dict(shared)
        xT = np.ascontiguousarray(numbers[cid * KLOC:(cid + 1) * KLOC, :].T, f)
        if cid == 0:
            # patch flat elements < 100 (k=0 column -> x row entries [t, 0])
            # so the constant affine reproduces their prefix normalization
            x_eff = x_norm0 / s_c + m_c
            xT[:STATS_CAP, 0] = x_eff
        xaug = np.ones((L, 2, KLOC), f)
        xaug[:, 0, :] = xT
        m["xT"] = xaug.astype(bf16)
        in_maps.append(m)
    return in_maps


def kernel(**inputs):
    if "nc" not in _compiled:
        _compiled["nc"] = _build()
    nc = _compiled["nc"]
    in_maps = _prep_inputs(**inputs)
    res = bass_utils.run_bass_kernel_spmd(nc, in_maps,
                                          core_ids=list(range(NCORES)))
    out = res.results[0]["out"]                            # [128, 2]
    return np.concatenate([out[:, 0], out[:, 1]])[None, :].astype(np.float32)


# revision 12
# speedup vs baseline: 1.0317x; 1.0063x over previous
"""JSONTreeLSTM Trainium2 kernel: 8-core data-parallel over K=4096 array children.

Layout: transposed — [128 partitions = mem/gate dims, K_loc=512 free = array index].
The number-embedding + running-stat normalization collapses algebraically into the
gate computation: gates = W_hh @ h + u' (x) x_raw_t + v', with
u' = s_c * (W_ih[:,128:] @ w_num), v' = W_ih[:,128:] @ b_num + b_ih + b_hh - m_c*u'
(s_c, m_c = the post-cap running stats, constant for all flat indices >= 100).
The 100 prefix-normalized elements (flat idx < 100 = numbers[0, :100], core 0 only)
are patched into x via x_eff = x_norm/s_c + m_c so the same affine maps them right.

Scan step (128 steps):
  DMA: stage x row t [1, 512] from DRAM to partition 0
  PE:  per gate g: psum_g = W_hh_g @ h.T + rank-1 u'_g (x) x_t
  ACT: sigma_g = Sigmoid(psum_g + v'_g)  (g-gate pre-scaled x2: tanh(g)=2*sig(2g)-1)
  DVE: w=2*sg2-1; m1=sf*c; m2=si*w; c2=m1+m2; ACT sig(2*c2); DVE w2=2*s-1; h2=so*w2
Root: sigmoid(W_fh h + b_fh)*c and h child-sums -> AllReduce -> tree-LSTM root.
"""
import sys

sys.path.insert(0, "/opt/trn_rl_repo")
import numpy as np
import concourse.bacc as bacc
import concourse.mybir as mybir
import concourse.tile as tile
from concourse import bass_utils

F32 = mybir.dt.float32
AF = mybir.ActivationFunctionType
OP = mybir.AluOpType
AX = mybir.AxisListType

K, L, MEM, NCORES = 4096, 128, 128, 8
KLOC = K // NCORES  # 512
STATS_CAP = 100

_compiled = {}


def _build(n_cores=NCORES):
    nc = bacc.Bacc("TRN2", target_bir_lowering=False, debug=False,
                   num_devices=n_cores)

    def din(name, shape):
        return nc.dram_tensor(name, shape, F32, kind="ExternalInput").ap()

    BF16 = mybir.dt.bfloat16
    xT_d = nc.dram_tensor("xT", [L, 2, KLOC], BF16,
                          kind="ExternalInput").ap()  # per t: [x_eff_t; ones]
    whhT_d = nc.dram_tensor("whhT", [MEM, 4 * MEM], BF16,
                            kind="ExternalInput").ap()  # W_hh.T, g-block x2
    uvrow_d = nc.dram_tensor("uvrow", [2, 4 * MEM], BF16,
                             kind="ExternalInput").ap()  # rows u', v', g-block x2
    wfhT_d = nc.dram_tensor("wfhT", [MEM, MEM], BF16, kind="ExternalInput").ap()
    bfh_d = din("bfh", [MEM, 1])
    wiouhT_d = din("wiouhT", [MEM, 3 * MEM])
    biouh_d = din("biouh", [MEM, 3])
    wloutT_d = din("wloutT", [MEM, MEM])
    blout_d = din("blout", [MEM, 1])
    out_d = nc.dram_tensor("out", [MEM, 2], F32, kind="ExternalOutput").ap()

    with tile.TileContext(nc) as tc:
        with tc.tile_pool(name="const", bufs=1) as cp, \
             tc.tile_pool(name="state", bufs=4) as sp, \
             tc.tile_pool(name="xrow", bufs=6) as xp, \
             tc.tile_pool(name="psum", bufs=2, space="PSUM") as pp, \
             tc.tile_pool(name="dram", bufs=1, space="DRAM") as dp:

            whhT = cp.tile([MEM, 4 * MEM], BF16, tag="whhT")
            uvrow = cp.tile([2, 4 * MEM], BF16, tag="uvrow")
            wfhT = cp.tile([MEM, MEM], BF16, tag="wfhT")
            bfh = cp.tile([MEM, 1], F32, tag="bfh")
            wiouhT = cp.tile([MEM, 3 * MEM], F32, tag="wiouhT")
            biouh = cp.tile([MEM, 3], F32, tag="biouh")
            wloutT = cp.tile([MEM, MEM], F32, tag="wloutT")
            blout = cp.tile([MEM, 1], F32, tag="blout")
            for t, d in [(whhT, whhT_d), (uvrow, uvrow_d),
                         (wfhT, wfhT_d), (bfh, bfh_d), (wiouhT, wiouhT_d),
                         (biouh, biouh_d), (wloutT, wloutT_d), (blout, blout_d)]:
                nc.sync.dma_start(t[:], d[:])

            # ---- LSTM scan: 2 independent k-chains hide the serial latency ----
            CH = 2
            KH = KLOC // CH
            h = []
            c = []
            for a in range(CH):
                ht = sp.tile([MEM, KH], BF16, tag=f"h{a}", name=f"h{a}_init")
                ct = sp.tile([MEM, KH], F32, tag=f"c{a}", name=f"c{a}_init")
                nc.any.memset(ht[:], 0.0)
                nc.any.memset(ct[:], 0.0)
                h.append(ht)
                c.append(ct)

            for t in range(L):
                xr = xp.tile([2, KLOC], BF16, tag="xr", name=f"xr_{t}")
                nc.sync.dma_start(xr[:], xT_d[t, :, :])
                # full-width x-injection: 4 matmuls over all k (both chains)
                gpf = pp.tile([MEM, 4 * KLOC], F32, tag="gpf", name=f"gpf_{t}")
                for j in range(4):
                    nc.tensor.matmul(gpf[:, j * KLOC:(j + 1) * KLOC],
                                     uvrow[:, j * MEM:(j + 1) * MEM],
                                     xr[:], start=True, stop=False)
                gpf3 = gpf[:].rearrange("p (g k) -> p g k", g=4)
                for a in range(CH):
                    ks = slice(a * KH, (a + 1) * KH)
                    sg = sp.tile([MEM, 4 * KH], F32, tag=f"sg{a}",
                                 name=f"sg{a}_{t}")
                    for j in range(4):
                        nc.tensor.matmul(gpf[:, j * KLOC + a * KH:
                                             j * KLOC + (a + 1) * KH],
                                         whhT[:, j * MEM:(j + 1) * MEM],
                                         h[a][:], start=False, stop=True)
                    sg3 = sg[:].rearrange("p (g k) -> p g k", g=4)
                    nc.scalar.activation(sg3, gpf3[:, :, ks], AF.Sigmoid)
                    si = sg[:, 0:KH]
                    sf = sg[:, KH:2 * KH]
                    sg2 = sg[:, 2 * KH:3 * KH]
                    so = sg[:, 3 * KH:4 * KH]
                    w = sp.tile([MEM, KH], F32, tag=f"w{a}", name=f"w{a}_{t}")
                    m1 = sp.tile([MEM, KH], F32, tag=f"m1{a}", name=f"m1{a}_{t}")
                    c2 = sp.tile([MEM, KH], F32, tag=f"c{a}", name=f"c{a}_{t}")
                    nc.vector.tensor_scalar(w, sg2, 2.0, -1.0,
                                            op0=OP.mult, op1=OP.add)
                    nc.vector.tensor_mul(m1, sf, c[a][:])
                    nc.vector.tensor_mul(w, si, w)
                    nc.vector.tensor_add(c2, m1, w)
                    s2c = sp.tile([MEM, KH], F32, tag=f"s2c{a}",
                                  name=f"s2c{a}_{t}")
                    nc.scalar.activation(s2c, c2[:], AF.Sigmoid, scale=2.0)
                    w2 = sp.tile([MEM, KH], F32, tag=f"w{a}", name=f"w2{a}_{t}")
                    h2 = sp.tile([MEM, KH], BF16, tag=f"h{a}", name=f"h{a}_{t}")
                    nc.vector.tensor_scalar(w2, s2c, 2.0, -1.0,
                                            op0=OP.mult, op1=OP.add)
                    nc.vector.tensor_mul(h2, so, w2)
                    h[a], c[a] = h2, c2

            # ---- root child-sum ----
            part4 = cp.tile([MEM, 4], F32, tag="part4")
            for a in range(CH):
                fgp = pp.tile([MEM, KH], F32, tag="gpf", name=f"fgp{a}")
                nc.tensor.matmul(fgp[:], wfhT[:], h[a][:], start=True, stop=True)
                fg = sp.tile([MEM, KH], F32, tag=f"sg{a}", name=f"fg{a}")
                nc.scalar.activation(fg, fgp[:], AF.Sigmoid, bias=bfh[:])
                fc = sp.tile([MEM, KH], F32, tag=f"w{a}", name=f"fc{a}")
                nc.vector.tensor_mul(fc, fg, c[a][:])
                nc.vector.reduce_sum(part4[:, a:a + 1], fc, axis=AX.X)
                nc.vector.reduce_sum(part4[:, 2 + a:3 + a], h[a][:], axis=AX.X)
            part = cp.tile([MEM, 2], F32, tag="part")
            nc.vector.tensor_add(part[:, 0:1], part4[:, 0:1], part4[:, 1:2])
            nc.vector.tensor_add(part[:, 1:2], part4[:, 2:3], part4[:, 3:4])

            bin_ = dp.tile([MEM, 2], F32)
            bout = dp.tile([MEM, 2], F32)
            nc.sync.dma_start(bin_[:], part[:])
            nc.gpsimd.collective_compute(
                "AllReduce", OP.add,
                replica_groups=[list(range(n_cores))],
                ins=[bin_.opt()], outs=[bout.opt()])
            red = cp.tile([MEM, 2], F32, tag="red")
            nc.sync.dma_start(red[:], bout[:])
            fcsum = red[:, 0:1]
            hbar = red[:, 1:2]

            # ---- root tree-LSTM ----
            ioup = pp.tile([MEM, 3], F32, tag="gpf")
            for j in range(3):
                nc.tensor.matmul(ioup[:, j:j + 1], wiouhT[:, j * MEM:(j + 1) * MEM],
                                 hbar, start=True, stop=True)
            rr = cp.tile([MEM, 8], F32, tag="rr")
            i_r = rr[:, 0:1]
            o_r = rr[:, 1:2]
            u_r = rr[:, 2:3]
            nc.scalar.activation(i_r, ioup[:, 0:1], AF.Sigmoid, bias=biouh[:, 0:1])
            nc.scalar.activation(o_r, ioup[:, 1:2], AF.Sigmoid, bias=biouh[:, 1:2])
            nc.scalar.activation(u_r, ioup[:, 2:3], AF.Tanh, bias=biouh[:, 2:3])
            cr = rr[:, 3:4]
            nc.vector.tensor_mul(cr, i_r, u_r)
            nc.vector.tensor_add(cr, cr, fcsum)
            tcr = rr[:, 4:5]
            nc.scalar.activation(tcr, cr, AF.Tanh)
            hr = rr[:, 5:6]
            nc.vector.tensor_mul(hr, o_r, tcr)
            hhp = pp.tile([MEM, 1], F32, tag="gpf")
            nc.tensor.matmul(hhp[:], wloutT[:], hr, start=True, stop=True)
            outs = cp.tile([MEM, 2], F32, tag="outs")
            nc.vector.tensor_copy(outs[:, 0:1], cr)
            nc.vector.tensor_scalar_add(outs[:, 1:2], hhp[:], blout[:])
            nc.sync.dma_start(out_d[:], outs[:])

    nc.compile()
    return nc


def _prep_inputs(numbers, w_num, b_num, W_ih, W_hh, b_ih, b_hh,
                 W_fh, b_fh, W_iouh, b_iouh, W_lout, b_lout):
    f = np.float32
    numbers = np.ascontiguousarray(numbers, f)

    # Running-stat normalization (reference semantics), first STATS_CAP elems.
    x100 = numbers.reshape(-1)[:STATS_CAP].astype(f)
    kk = np.arange(1, STATS_CAP + 1, dtype=f)
    cs = np.cumsum(x100, dtype=f)
    css = np.cumsum(x100 * x100, dtype=f)
    mean_k = cs / kk
    var_k = np.maximum(css / kk - mean_k * mean_k, 0.0)
    std_k = np.sqrt(var_k)
    use_k = (kk > 3.0) & (std_k > 1e-8)
    inv_k = np.where(use_k, 1.0 / np.where(use_k, std_k, 1.0), 1.0).astype(f)
    x_norm0 = (x100 - mean_k) * inv_k
    m_c = float(mean_k[-1])
    s_c = float(inv_k[-1])

    Wr = np.asarray(W_ih, f)[:, MEM:]                      # [512, 128]
    u = (Wr @ np.asarray(w_num, f)) * s_c
    v = (Wr @ np.asarray(b_num, f) + np.asarray(b_ih, f)
         + np.asarray(b_hh, f) - m_c * u)
    whhT = np.asarray(W_hh, f).T.copy()                    # [128, 512]
    whhT[:, 2 * MEM:3 * MEM] *= 2.0                        # g-gate -> 2g
    u = u.astype(f).copy()
    v = v.astype(f).copy()
    u[2 * MEM:3 * MEM] *= 2.0
    v[2 * MEM:3 * MEM] *= 2.0

    try:
        import ml_dtypes
        bf16 = ml_dtypes.bfloat16
    except ImportError:
        import jax.numpy as jnp
        bf16 = jnp.bfloat16
    shared = {
        "whhT": whhT.astype(bf16),
        "uvrow": np.stack([u, v]).astype(bf16),
        "wfhT": np.asarray(W_fh, f).T.copy().astype(bf16),
        "bfh": np.asarray(b_fh, f).reshape(MEM, 1).copy(),
        "wiouhT": np.asarray(W_iouh, f).T.copy(),
        "biouh": np.asarray(b_iouh, f).reshape(3, MEM).T.copy(),
        "wloutT": np.asarray(W_lout, f).T.copy(),
        "blout": np.asarray(b_lout, f).reshape(MEM, 1).copy(),
    }
    in_maps = []
    for cid in range(NCORES):
        m = dict(shared)
        xT = np.ascontiguousarray(numbers[cid * KLOC:(cid + 1) * KLOC, :].T, f)
        if cid == 0:
            # patch flat elements < 100 (k=0 column -> x row entries [t, 0])
            # so the constant affine reproduces their prefix normalization
            x_eff = x_norm0 / s_c + m_c
            xT[:STATS_CAP, 0] = x_eff
        xaug = np.ones((L, 2, KLOC), f)
        xaug[:, 0, :] = xT
        m["xT"] = xaug.astype(bf16)
        in_maps.append(m)
    return in_maps


def kernel(**inputs):
    if "nc" not in _compiled:
        _compiled["nc"] = _build()
    nc = _compiled["nc"]
    in_maps = _prep_inputs(**inputs)
    res = bass_utils.run_bass_kernel_spmd(nc, in_maps,
                                          core_ids=list(range(NCORES)))
    out = res.results[0]["out"]                            # [128, 2]
    return np.concatenate([out[:, 0], out[:, 1]])[None, :].astype(np.float32)


# revision 13
# speedup vs baseline: 1.0433x; 1.0112x over previous
"""JSONTreeLSTM Trainium2 kernel: 8-core data-parallel over K=4096 array children.

Layout: transposed — [128 partitions = mem/gate dims, K_loc=512 free = array index].
The number-embedding + running-stat normalization collapses algebraically into the
gate computation: gates = W_hh @ h + u' (x) x_raw_t + v', with
u' = s_c * (W_ih[:,128:] @ w_num), v' = W_ih[:,128:] @ b_num + b_ih + b_hh - m_c*u'
(s_c, m_c = the post-cap running stats, constant for all flat indices >= 100).
The 100 prefix-normalized elements (flat idx < 100 = numbers[0, :100], core 0 only)
are patched into x via x_eff = x_norm/s_c + m_c so the same affine maps them right.

Scan step (128 steps):
  DMA: stage x row t [1, 512] from DRAM to partition 0
  PE:  per gate g: psum_g = W_hh_g @ h.T + rank-1 u'_g (x) x_t
  ACT: sigma_g = Sigmoid(psum_g + v'_g)  (g-gate pre-scaled x2: tanh(g)=2*sig(2g)-1)
  DVE: w=2*sg2-1; m1=sf*c; m2=si*w; c2=m1+m2; ACT sig(2*c2); DVE w2=2*s-1; h2=so*w2
Root: sigmoid(W_fh h + b_fh)*c and h child-sums -> AllReduce -> tree-LSTM root.
"""
import sys

sys.path.insert(0, "/opt/trn_rl_repo")
import numpy as np
import concourse.bacc as bacc
import concourse.mybir as mybir
import concourse.tile as tile
from concourse import bass_utils

F32 = mybir.dt.float32
AF = mybir.ActivationFunctionType
OP = mybir.AluOpType
AX = mybir.AxisListType

K, L, MEM, NCORES = 4096, 128, 128, 8
KLOC = K // NCORES  # 512
STATS_CAP = 100

_compiled = {}


def _build(n_cores=NCORES):
    nc = bacc.Bacc("TRN2", target_bir_lowering=False, debug=False,
                   num_devices=n_cores)

    def din(name, shape):
        return nc.dram_tensor(name, shape, F32, kind="ExternalInput").ap()

    BF16 = mybir.dt.bfloat16
    xT_d = nc.dram_tensor("xT", [L, 2, KLOC], BF16,
                          kind="ExternalInput").ap()  # per t: [x_eff_t; ones]
    whhT_d = nc.dram_tensor("whhT", [MEM, 4 * MEM], BF16,
                            kind="ExternalInput").ap()  # W_hh.T, g-block x2
    uvrow_d = nc.dram_tensor("uvrow", [2, 4 * MEM], BF16,
                             kind="ExternalInput").ap()  # rows u', v', g-block x2
    wfhT_d = nc.dram_tensor("wfhT", [MEM, MEM], BF16, kind="ExternalInput").ap()
    bfh_d = din("bfh", [MEM, 1])
    wiouhT_d = din("wiouhT", [MEM, 3 * MEM])
    biouh_d = din("biouh", [MEM, 3])
    wloutT_d = din("wloutT", [MEM, MEM])
    blout_d = din("blout", [MEM, 1])
    out_d = nc.dram_tensor("out", [MEM, 2], F32, kind="ExternalOutput").ap()

    with tile.TileContext(nc) as tc:
        with tc.tile_pool(name="const", bufs=1) as cp, \
             tc.tile_pool(name="state", bufs=4) as sp, \
             tc.tile_pool(name="xrow", bufs=6) as xp, \
             tc.tile_pool(name="psum", bufs=2, space="PSUM") as pp, \
             tc.tile_pool(name="dram", bufs=1, space="DRAM") as dp:

            whhT = cp.tile([MEM, 4 * MEM], BF16, tag="whhT")
            uvrow = cp.tile([2, 4 * MEM], BF16, tag="uvrow")
            wfhT = cp.tile([MEM, MEM], BF16, tag="wfhT")
            bfh = cp.tile([MEM, 1], F32, tag="bfh")
            wiouhT = cp.tile([MEM, 3 * MEM], F32, tag="wiouhT")
            biouh = cp.tile([MEM, 3], F32, tag="biouh")
            wloutT = cp.tile([MEM, MEM], F32, tag="wloutT")
            blout = cp.tile([MEM, 1], F32, tag="blout")
            for t, d in [(whhT, whhT_d), (uvrow, uvrow_d),
                         (wfhT, wfhT_d), (bfh, bfh_d), (wiouhT, wiouhT_d),
                         (biouh, biouh_d), (wloutT, wloutT_d), (blout, blout_d)]:
                nc.sync.dma_start(t[:], d[:])

            # ---- LSTM scan: 2 independent k-chains hide the serial latency ----
            CH = 2
            KH = KLOC // CH
            h = []
            c = []
            for a in range(CH):
                ht = sp.tile([MEM, KH], BF16, tag=f"h{a}", name=f"h{a}_init")
                ct = sp.tile([MEM, KH], F32, tag=f"c{a}", name=f"c{a}_init")
                nc.any.memset(ht[:], 0.0)
                nc.any.memset(ct[:], 0.0)
                h.append(ht)
                c.append(ct)

            for t in range(L):
                xr = xp.tile([2, KLOC], BF16, tag="xr", name=f"xr_{t}")
                nc.sync.dma_start(xr[:], xT_d[t, :, :])
                # full-width x-injection: 4 matmuls over all k (both chains)
                gpf = pp.tile([MEM, 4 * KLOC], F32, tag="gpf", name=f"gpf_{t}")
                for j in range(4):
                    nc.tensor.matmul(gpf[:, j * KLOC:(j + 1) * KLOC],
                                     uvrow[:, j * MEM:(j + 1) * MEM],
                                     xr[:], start=True, stop=False)
                gpf3 = gpf[:].rearrange("p (g k) -> p g k", g=4)
                for a in range(CH):
                    ks = slice(a * KH, (a + 1) * KH)
                    sg = sp.tile([MEM, 4 * KH], F32, tag=f"sg{a}",
                                 name=f"sg{a}_{t}")
                    for j in range(4):
                        nc.tensor.matmul(gpf[:, j * KLOC + a * KH:
                                             j * KLOC + (a + 1) * KH],
                                         whhT[:, j * MEM:(j + 1) * MEM],
                                         h[a][:], start=False, stop=True)
                    sg3 = sg[:].rearrange("p (g k) -> p g k", g=4)
                    nc.scalar.activation(sg3, gpf3[:, :, ks], AF.Sigmoid)
                    si = sg[:, 0:KH]
                    sf = sg[:, KH:2 * KH]
                    sg2 = sg[:, 2 * KH:3 * KH]
                    so = sg[:, 3 * KH:4 * KH]
                    w = sp.tile([MEM, KH], F32, tag=f"w{a}", name=f"w{a}_{t}")
                    m1 = sp.tile([MEM, KH], F32, tag=f"m1{a}", name=f"m1{a}_{t}")
                    c2 = sp.tile([MEM, KH], F32, tag=f"c{a}", name=f"c{a}_{t}")
                    nc.vector.tensor_scalar(w, sg2, 2.0, -1.0,
                                            op0=OP.mult, op1=OP.add)
                    nc.vector.tensor_mul(m1, sf, c[a][:])
                    nc.vector.tensor_mul(w, si, w)
                    nc.vector.tensor_add(c2, m1, w)
                    s2c = sp.tile([MEM, KH], F32, tag=f"s2c{a}",
                                  name=f"s2c{a}_{t}")
                    nc.scalar.activation(s2c, c2[:], AF.Sigmoid, scale=2.0)
                    w2 = sp.tile([MEM, KH], F32, tag=f"w{a}", name=f"w2{a}_{t}")
                    h2 = sp.tile([MEM, KH], BF16, tag=f"h{a}", name=f"h{a}_{t}")
                    nc.vector.tensor_scalar(w2, s2c, 2.0, -1.0,
                                            op0=OP.mult, op1=OP.add)
                    nc.vector.tensor_mul(h2, so, w2)
                    h[a], c[a] = h2, c2

            # ---- root child-sum ----
            part4 = cp.tile([MEM, 4], F32, tag="part4")
            for a in range(CH):
                fgp = pp.tile([MEM, KH], F32, tag="gpf", name=f"fgp{a}")
                nc.tensor.matmul(fgp[:], wfhT[:], h[a][:], start=True, stop=True)
                fg = sp.tile([MEM, KH], F32, tag=f"sg{a}", name=f"fg{a}")
                nc.scalar.activation(fg, fgp[:], AF.Sigmoid, bias=bfh[:])
                fc = sp.tile([MEM, KH], F32, tag=f"w{a}", name=f"fc{a}")
                nc.vector.tensor_mul(fc, fg, c[a][:])
                nc.vector.reduce_sum(part4[:, a:a + 1], fc, axis=AX.X)
                nc.vector.reduce_sum(part4[:, 2 + a:3 + a], h[a][:], axis=AX.X)
            part = cp.tile([MEM, 2], F32, tag="part")
            nc.vector.tensor_add(part[:, 0:1], part4[:, 0:1], part4[:, 1:2])
            nc.vector.tensor_add(part[:, 1:2], part4[:, 2:3], part4[:, 3:4])

            bin_ = dp.tile([MEM, 2], F32)
            bout = dp.tile([MEM, 2], F32)
            nc.sync.dma_start(bin_[:], part[:])
            nc.gpsimd.collective_compute(
                "AllReduce", OP.add,
                replica_groups=[list(range(n_cores))],
                ins=[bin_.opt()], outs=[bout.opt()])
            red = cp.tile([MEM, 2], F32, tag="red")
            nc.sync.dma_start(red[:], bout[:])
            fcsum = red[:, 0:1]
            hbar = red[:, 1:2]

            # ---- root tree-LSTM ----
            ioup = pp.tile([MEM, 3], F32, tag="gpf")
            for j in range(3):
                nc.tensor.matmul(ioup[:, j:j + 1], wiouhT[:, j * MEM:(j + 1) * MEM],
                                 hbar, start=True, stop=True)
            rr = cp.tile([MEM, 8], F32, tag="rr")
            i_r = rr[:, 0:1]
            o_r = rr[:, 1:2]
            u_r = rr[:, 2:3]
            nc.scalar.activation(i_r, ioup[:, 0:1], AF.Sigmoid, bias=biouh[:, 0:1])
            nc.scalar.activation(o_r, ioup[:, 1:2], AF.Sigmoid, bias=biouh[:, 1:2])
            nc.scalar.activation(u_r, ioup[:, 2:3], AF.Tanh, bias=biouh[:, 2:3])
            cr = rr[:, 3:4]
            nc.vector.tensor_mul(cr, i_r, u_r)
            nc.vector.tensor_add(cr, cr, fcsum)
            tcr = rr[:, 4:5]
            nc.scalar.activation(tcr, cr, AF.Tanh)
            hr = rr[:, 5:6]
            nc.vector.tensor_mul(hr, o_r, tcr)
            hhp = pp.tile([MEM, 1], F32, tag="gpf")
            nc.tensor.matmul(hhp[:], wloutT[:], hr, start=True, stop=True)
            outs = cp.tile([MEM, 2], F32, tag="outs")
            nc.vector.tensor_copy(outs[:, 0:1], cr)
            nc.vector.tensor_scalar_add(outs[:, 1:2], hhp[:], blout[:])
            nc.sync.dma_start(out_d[:], outs[:])

    nc.compile()
    return nc


def _prep_inputs(numbers, w_num, b_num, W_ih, W_hh, b_ih, b_hh,
                 W_fh, b_fh, W_iouh, b_iouh, W_lout, b_lout):
    f = np.float32
    numbers = np.ascontiguousarray(numbers, f)

    # Running-stat normalization (reference semantics), first STATS_CAP elems.
    x100 = numbers.reshape(-1)[:STATS_CAP].astype(f)
    kk = np.arange(1, STATS_CAP + 1, dtype=f)
    cs = np.cumsum(x100, dtype=f)
    css = np.cumsum(x100 * x100, dtype=f)
    mean_k = cs / kk
    var_k = np.maximum(css / kk - mean_k * mean_k, 0.0)
    std_k = np.sqrt(var_k)
    use_k = (kk > 3.0) & (std_k > 1e-8)
    inv_k = np.where(use_k, 1.0 / np.where(use_k, std_k, 1.0), 1.0).astype(f)
    x_norm0 = (x100 - mean_k) * inv_k
    m_c = float(mean_k[-1])
    s_c = float(inv_k[-1])

    Wr = np.asarray(W_ih, f)[:, MEM:]                      # [512, 128]
    u = (Wr @ np.asarray(w_num, f)) * s_c
    v = (Wr @ np.asarray(b_num, f) + np.asarray(b_ih, f)
         + np.asarray(b_hh, f) - m_c * u)
    whhT = np.asarray(W_hh, f).T.copy()                    # [128, 512]
    whhT[:, 2 * MEM:3 * MEM] *= 2.0                        # g-gate -> 2g
    u = u.astype(f).copy()
    v = v.astype(f).copy()
    u[2 * MEM:3 * MEM] *= 2.0
    v[2 * MEM:3 * MEM] *= 2.0

    try:
        import ml_dtypes
        bf16 = ml_dtypes.bfloat16
    except ImportError:
        import jax.numpy as jnp
        bf16 = jnp.bfloat16
    shared = {
        "whhT": whhT.astype(bf16),
        "uvrow": np.stack([u, v]).astype(bf16),
        "wfhT": np.asarray(W_fh, f).T.copy().astype(bf16),
        "bfh": np.asarray(b_fh, f).reshape(MEM, 1).copy(),
        "wiouhT": np.asarray(W_iouh, f).T.copy(),
        "biouh": np.asarray(b_iouh, f).reshape(3, MEM).T.copy(),
        "wloutT": np.asarray(W_lout, f).T.copy(),
        "blout": np.asarray(b_lout, f).reshape(MEM, 1).copy(),
    }
    in_maps = []
    for cid in range(NCORES):
        m = dict(shared)
        xT = np.ascontiguousarray(numbers[cid * KLOC:(cid + 1) * KLOC, :].T, f)
        if cid == 0:
            # patch flat elements < 100 (k=0 column -> x row entries [t, 0])
            # so the constant affine reproduces their prefix normalization
            x_eff = x_norm0 / s_c + m_c
            xT[:STATS_CAP, 0] = x_eff
        xaug = np.ones((L, 2, KLOC), f)
        xaug[:, 0, :] = xT
        m["xT"] = xaug.astype(bf16)
        in_maps.append(m)
    return in_maps


def kernel(**inputs):
    if "nc" not in _compiled:
        _compiled["nc"] = _build()
    nc = _compiled["nc"]
    in_maps = _prep_inputs(**inputs)
    last_err = None
    for _attempt in range(3):
        try:
            res = bass_utils.run_bass_kernel_spmd(nc, in_maps,
                                                  core_ids=list(range(NCORES)))
            break
        except Exception as e:  # transient NRT device faults happen rarely
            last_err = e
    else:
        raise last_err
    out = res.results[0]["out"]                            # [128, 2]
    return np.concatenate([out[:, 0], out[:, 1]])[None, :].astype(np.float32)
